# revision 11
# baseline (speedup 1.0000x reference)
"""GCN (5x GCNConv + 2-layer head) on 8 Trainium2 NeuronCores.

Strategy:
  - Node sharding: each core owns NPC = tiles_pc*128 node rows (dst tiles).
  - Per layer: transform (x @ W, float32r matmuls) -> AllGather of h (bf16)
    -> aggregation as one-hot matmul scatter (bf16) fused with relu,
    PE-transpose and the next layer's transform.
  - Edge gather via dma_gather (SWDGE) with lo/hi source-range split
    (int16 index limit of 32768 rows per gather table slice).
"""

import math
import numpy as np
import ml_dtypes

P = 128
NCORES = 8
LOBASE = 32768  # max rows addressable by one int16 gather table slice


# ---------------------------------------------------------------- host planning

def _plan(edge_index, n_nodes, tiles_pc, gblk, lobase):
    """Build the (schedule, per-core data) for the aggregation.

    Returns meta (identical across cores; baked into the program) and
    per-core numpy inputs (M matrices + wrapped int16 gather indices).
    """
    npc = tiles_pc * P
    npad = NCORES * npc
    n_tiles_g = NCORES * tiles_pc

    src = np.concatenate([edge_index[0], np.arange(n_nodes, dtype=np.int64)])
    dst = np.concatenate([edge_index[1], np.arange(n_nodes, dtype=np.int64)])
    deg = np.bincount(dst, minlength=npad).astype(np.float64)
    deg[deg == 0] = 1.0
    dinv = 1.0 / np.sqrt(deg)
    norm = (dinv[src] * dinv[dst]).astype(np.float32)

    gtile = (dst // P).astype(np.int64)          # global dst tile
    core = gtile // tiles_pc
    ltile = gtile % tiles_pc                     # tile position within core
    rng_ = (src >= lobase).astype(np.int64)      # 0 = lo, 1 = hi

    # counts per (core, ltile, range)
    cnt = np.zeros((NCORES, tiles_pc, 2), np.int64)
    np.add.at(cnt, (core, ltile, rng_), 1)
    kch = np.ceil(cnt / P).astype(np.int64).max(axis=0)  # [tiles_pc, 2]
    kch[:, 0] = np.maximum(kch[:, 0], 1)  # every tile has >=1 chunk (stop flag)
    if lobase >= npad:
        kch[:, 1] = 0  # no hi range

    # block partition of tile positions
    blocks = [(b, min(b + gblk, tiles_pc)) for b in range(0, tiles_pc, gblk)]

    # slot layout: for blk: lo slots of its tiles, then hi slots
    slot_base = {}   # (t, r) -> first slot
    gathers = []     # per blk: list of (r, slot0, nslots, col0)
    s = 0
    col = 0
    for (t0, t1) in blocks:
        gspec = []
        for r in (0, 1):
            s0, c0 = s, col
            for t in range(t0, t1):
                slot_base[(t, r)] = s
                s += int(kch[t, r])
            nsl = s - s0
            gspec.append((r, s0, nsl, c0))
            col += nsl * P // 16
        gathers.append(gspec)
    s_total = s
    idx_cols = col

    # chunk list per tile position: [(slot, r), ...]
    chunks_per_tile = []
    for t in range(tiles_pc):
        lst = [(slot_base[(t, 0)] + j, 0) for j in range(int(kch[t, 0]))]
        lst += [(slot_base[(t, 1)] + j, 1) for j in range(int(kch[t, 1]))]
        chunks_per_tile.append(lst)

    # block-local position of each slot inside its block's gather output
    slot_blk_pos = np.zeros(s_total, np.int64)
    blk_of_slot = np.zeros(s_total, np.int64)
    blk_slots = []
    for bi, gspec in enumerate(gathers):
        base = gspec[0][1]
        nsl = gspec[0][2] + gspec[1][2]
        blk_slots.append(nsl)
        for j in range(nsl):
            slot_blk_pos[base + j] = j
            blk_of_slot[base + j] = bi

    meta = dict(
        tiles_pc=tiles_pc, npc=npc, npad=npad, blocks=blocks, gathers=gathers,
        chunks_per_tile=chunks_per_tile, s_total=s_total, idx_cols=idx_cols,
        blk_slots=blk_slots, kch=kch,
    )

    # ---- per-core data: M [s_total, P, P] bf16 and IDXW [128, idx_cols] int16
    order = np.lexsort((src, rng_, ltile, core))
    so, do, no, co, to, ro = (a[order] for a in (src, dst, norm, core, ltile, rng_))
    # position of each edge within its (core, tile, range) group
    key = ((co * tiles_pc + to) * 2 + ro)
    grp_start = np.searchsorted(key, np.arange(n_tiles_g * 2, dtype=np.int64), side="left")
    pos = np.arange(len(so)) - grp_start[key]
    chunk_j = pos // P
    eloc = pos % P
    base_slot = np.array([[slot_base[(t, r)] for r in (0, 1)] for t in range(tiles_pc)])
    slot = base_slot[to, ro] + chunk_j
    dloc = (do % P).astype(np.int64)
    idx_val = (so - ro * lobase).astype(np.int16)

    colbase = np.zeros(s_total, np.int64)
    blkpos0 = np.zeros(s_total, np.int64)
    for gspec in gathers:
        for (r, s0, nsl, c0) in gspec:
            colbase[s0:s0 + nsl] = c0
            blkpos0[s0:s0 + nsl] = s0  # gather-local base slot

    m_cores, idx_cores = [], []
    for c in range(NCORES):
        sel = co == c
        M = np.zeros((s_total, P, P), ml_dtypes.bfloat16)
        M[slot[sel], eloc[sel], dloc[sel]] = no[sel].astype(ml_dtypes.bfloat16)
        idxw = np.zeros((16, idx_cols), np.int16)
        j = (slot[sel] - blkpos0[slot[sel]]) * P + eloc[sel]
        idxw[j % 16, colbase[slot[sel]] + j // 16] = idx_val[sel]
        m_cores.append(M)
        idx_cores.append(np.tile(idxw, (8, 1)))
    return meta, m_cores, idx_cores


# ---------------------------------------------------------------- device kernel

def _build(meta, F, H1, H2, UH, OUT, lobase):
    import concourse.bass as bass
    import concourse.bacc as bacc
    import concourse.tile as tile
    import concourse.mybir as mybir

    dt = mybir.dt
    AF = mybir.ActivationFunctionType
    tiles_pc = meta["tiles_pc"]
    npc, npad = meta["npc"], meta["npad"]
    s_total, idx_cols = meta["s_total"], meta["idx_cols"]
    WIDTH = [H1, H2, H2, H2, H2]          # width of h_l, l=1..5
    WMAT_IN = [F, H1, H2, H2, H2]         # contraction dim of W_l

    nc = bacc.Bacc("TRN2", target_bir_lowering=False, debug=False,
                   num_devices=NCORES)

    xT = nc.dram_tensor("xT", [F, npc], dt.float32r, kind="ExternalInput").ap()
    Ws = [nc.dram_tensor(f"W{l+1}", [WMAT_IN[l], WIDTH[l]], dt.float32r,
                         kind="ExternalInput").ap() for l in range(5)]
    FBs = [nc.dram_tensor(f"FB{l+1}", [P, WIDTH[l]], dt.bfloat16,
                          kind="ExternalInput").ap() for l in range(5)]
    Uw1 = nc.dram_tensor("Uw1", [H2, UH], dt.float32r, kind="ExternalInput").ap()
    Uw2 = nc.dram_tensor("Uw2", [UH, OUT], dt.float32r, kind="ExternalInput").ap()
    Ub1 = nc.dram_tensor("Ub1", [UH, 1], dt.float32, kind="ExternalInput").ap()
    Ub2 = nc.dram_tensor("Ub2", [OUT, 1], dt.float32, kind="ExternalInput").ap()
    Mt = nc.dram_tensor("M", [s_total, P, P], dt.bfloat16, kind="ExternalInput").ap()
    IDXW = nc.dram_tensor("IDXW", [P, idx_cols], dt.int16, kind="ExternalInput").ap()
    IDENT = nc.dram_tensor("IDENT", [P, P], dt.float32r, kind="ExternalInput").ap()
    MBIAS = nc.dram_tensor("MBIAS", [P, P], dt.bfloat16, kind="ExternalInput").ap()
    outT = nc.dram_tensor("outT", [OUT, npc], dt.float32, kind="ExternalOutput").ap()

    with tile.TileContext(nc) as tc:
        with tc.tile_pool(name="res", bufs=1) as res, \
             tc.tile_pool(name="dram", bufs=1, space="DRAM") as dram:
            # resident tiles
            m_sb = res.tile([P, s_total * P], dt.bfloat16)
            nc.sync.dma_start(
                m_sb[:].rearrange("e (s d) -> e s d", d=P),
                Mt[:].rearrange("s e d -> e s d"))
            idx_sb = res.tile([P, idx_cols], dt.int16)
            nc.sync.dma_start(idx_sb[:], IDXW[:])
            ident_sb = res.tile([P, P], dt.float32r)
            nc.sync.dma_start(ident_sb[:], IDENT[:])
            mbias_sb = res.tile([P, P], dt.bfloat16)
            nc.sync.dma_start(mbias_sb[:], MBIAS[:])
            fb_sb = []
            for l in range(5):
                t = res.tile([P, WIDTH[l]], dt.bfloat16, name=f"fb{l}")
                nc.sync.dma_start(t[:], FBs[l][:])
                fb_sb.append(t)
            uw1_sb = res.tile([P, (H2 // P) * UH], dt.float32r)
            nc.sync.dma_start(uw1_sb[:].rearrange("p (k u) -> p k u", u=UH),
                              Uw1[:].rearrange("(k p) u -> p k u", p=P))
            uw2_sb = res.tile([UH, OUT], dt.float32r)
            nc.sync.dma_start(uw2_sb[:], Uw2[:])
            ub1_sb = res.tile([UH, 1], dt.float32)
            nc.sync.dma_start(ub1_sb[:], Ub1[:])
            ub2_sb = res.tile([OUT, 1], dt.float32)
            nc.sync.dma_start(ub2_sb[:], Ub2[:])

            h_own = [dram.tile([npc, WIDTH[l]], dt.bfloat16, name=f"h_own{l}")
                     for l in range(5)]
            h_full = [dram.tile([npad, WIDTH[l]], dt.bfloat16, name=f"h_full{l}")
                      for l in range(5)]

            def allgather(l):
                nc.gpsimd.collective_compute(
                    "AllGather", mybir.AluOpType.bypass,
                    replica_groups=[list(range(NCORES))],
                    ins=[h_own[l][:]], outs=[h_full[l][:]])

            # ---------------- stage 0: transform layer 1 (x @ W1)
            kt1 = F // P
            GW = 4  # tiles per xT column group
            with tc.tile_pool(name="s0w", bufs=1) as s0w, \
                 tc.tile_pool(name="s0x", bufs=2) as s0x, \
                 tc.tile_pool(name="s0ps", bufs=2, space="PSUM") as s0ps, \
                 tc.tile_pool(name="s0h", bufs=3) as s0h:
                w1_sb = s0w.tile([P, kt1 * H1], dt.float32r)
                nc.sync.dma_start(
                    w1_sb[:].rearrange("p (k h) -> p k h", h=H1),
                    Ws[0][:].rearrange("(k p) h -> p k h", p=P))
                w1r = w1_sb[:].rearrange("p (k h) -> p k h", h=H1)
                for g0 in range(0, tiles_pc, GW):
                    g1 = min(g0 + GW, tiles_pc)
                    gw = (g1 - g0) * P
                    xg = s0x.tile([P, kt1 * GW * P], dt.float32r, tag="xg")
                    nc.sync.dma_start(
                        xg[:, :kt1 * gw].rearrange("p (k c) -> p k c", c=gw),
                        xT[:, g0 * P:g1 * P].rearrange("(k p) c -> p k c", p=P))
                    xgr = xg[:, :kt1 * gw].rearrange("p (k c) -> p k c", c=gw)
                    for t in range(g0, g1):
                        tl = (t - g0) * P
                        ps = s0ps.tile([P, H1], dt.float32, space="PSUM", tag="ps")
                        nreg0 = max(H1 // 512, 1)
                        rw0 = min(H1, 512)
                        for k in range(kt1):
                            for nh in range(nreg0):
                                nc.tensor.matmul(
                                    ps[:, nh * rw0:(nh + 1) * rw0],
                                    lhsT=xgr[:, k, tl:tl + P],
                                    rhs=w1r[:, k, nh * rw0:(nh + 1) * rw0],
                                    start=(k == 0), stop=(k == kt1 - 1))
                        hb = s0h.tile([P, H1], dt.bfloat16, tag="hb")
                        nc.scalar.activation(hb[:], ps[:], AF.Copy)
                        nc.sync.dma_start(h_own[0][t * P:(t + 1) * P, :], hb[:])
            allgather(0)

            # ---------------- stages l=1..5: agg + (next transform | head)
            for l in range(1, 6):
                w_in = WIDTH[l - 1]
                ktin = w_in // P
                nreg = max(w_in // 512, 1)
                rwid = min(w_in, 512)
                w_out = WIDTH[l] if l < 5 else None
                with tc.tile_pool(name=f"s{l}w", bufs=1) as sw, \
                     tc.tile_pool(name=f"s{l}g", bufs=2) as sg, \
                     tc.tile_pool(name=f"s{l}ag", bufs=2, space="PSUM") as agp, \
                     tc.tile_pool(name=f"s{l}tr", bufs=2, space="PSUM") as trp, \
                     tc.tile_pool(name=f"s{l}tf", bufs=2, space="PSUM") as tfp, \
                     tc.tile_pool(name=f"s{l}sb", bufs=3) as sb:
                    if l < 5:
                        wn_sb = sw.tile([P, ktin * w_out], dt.float32r)
                        nc.sync.dma_start(
                            wn_sb[:].rearrange("p (k h) -> p k h", h=w_out),
                            Ws[l][:].rearrange("(k p) h -> p k h", p=P))
                        wnr = wn_sb[:].rearrange("p (k h) -> p k h", h=w_out)
                    uw1r = uw1_sb[:].rearrange("p (k u) -> p k u", u=UH)

                    for bi, (t0, t1) in enumerate(meta["blocks"]):
                        nsl = meta["blk_slots"][bi]
                        gt = sg.tile([P, nsl * w_in], dt.bfloat16, tag="gt")
                        g3 = gt[:].rearrange("p (s e) -> p s e", e=w_in)
                        for (r, s0, ns_r, c0) in meta["gathers"][bi]:
                            if ns_r == 0:
                                continue
                            pos0 = s0 - meta["gathers"][bi][0][1]
                            tslice = (h_full[l - 1][0:lobase, :] if r == 0
                                      else h_full[l - 1][lobase:npad, :])
                            nc.gpsimd.dma_gather(
                                out_ap=g3[:, pos0:pos0 + ns_r, :],
                                in_ap=tslice,
                                idxs_ap=idx_sb[:, c0:c0 + ns_r * P // 16],
                                num_idxs=ns_r * P, num_idxs_reg=ns_r * P,
                                elem_size=w_in)
                        for t in range(t0, t1):
                            ps = agp.tile([P, w_in], dt.float32, space="PSUM",
                                          tag="agps")
                            chunks = meta["chunks_per_tile"][t]
                            for nh in range(nreg):
                                nc.tensor.matmul(
                                    ps[:, nh * rwid:(nh + 1) * rwid],
                                    lhsT=mbias_sb[:],
                                    rhs=fb_sb[l - 1][:, nh * rwid:(nh + 1) * rwid],
                                    start=True, stop=False)
                            for ci, (s, r) in enumerate(chunks):
                                pos = meta["gathers"][bi][0][1]
                                sl = s - pos
                                last = ci == len(chunks) - 1
                                for nh in range(nreg):
                                    nc.tensor.matmul(
                                        ps[:, nh * rwid:(nh + 1) * rwid],
                                        lhsT=m_sb[:, s * P:(s + 1) * P],
                                        rhs=g3[:, sl, nh * rwid:(nh + 1) * rwid],
                                        start=False, stop=last)
                            xr = sb.tile([P, w_in], dt.float32r, tag="xr")
                            nc.scalar.activation(xr[:], ps[:], AF.Relu)
                            xt = sb.tile([P, w_in], dt.float32r, tag="xt")
                            for k in range(ktin):
                                tp = trp.tile([P, P], dt.float32r, space="PSUM",
                                              tag="trps")
                                nc.tensor.transpose(
                                    tp[:], xr[:, k * P:(k + 1) * P], ident_sb[:])
                                nc.vector.tensor_copy(xt[:, k * P:(k + 1) * P], tp[:])
                            if l < 5:
                                ps2 = tfp.tile([P, w_out], dt.float32,
                                               space="PSUM", tag="tfps")
                                for k in range(ktin):
                                    nc.tensor.matmul(
                                        ps2[:], lhsT=xt[:, k * P:(k + 1) * P],
                                        rhs=wnr[:, k, :],
                                        start=(k == 0), stop=(k == ktin - 1))
                                hb = sb.tile([P, w_out], dt.bfloat16, tag="hb")
                                nc.scalar.activation(hb[:], ps2[:], AF.Copy)
                                nc.sync.dma_start(
                                    h_own[l][t * P:(t + 1) * P, :], hb[:])
                            else:
                                psh = tfp.tile([UH, P], dt.float32, space="PSUM",
                                               tag="h1ps")
                                for k in range(ktin):
                                    nc.tensor.matmul(
                                        psh[:], lhsT=uw1r[:, k, :],
                                        rhs=xt[:, k * P:(k + 1) * P],
                                        start=(k == 0), stop=(k == ktin - 1))
                                hh = sb.tile([UH, P], dt.float32r, tag="hh")
                                nc.scalar.activation(hh[:], psh[:], AF.Relu,
                                                     bias=ub1_sb[:])
                                pso = trp.tile([OUT, P], dt.float32, space="PSUM",
                                               tag="h2ps")
                                nc.tensor.matmul(pso[:], lhsT=uw2_sb[:],
                                                 rhs=hh[:], start=True, stop=True)
                                ob = sb.tile([OUT, P], dt.float32, tag="ob")
                                nc.vector.tensor_tensor(
                                    ob[:], pso[:], ub2_sb[:].to_broadcast([OUT, P]),
                                    op=mybir.AluOpType.add)
                                nc.sync.dma_start(outT[:, t * P:(t + 1) * P], ob[:])
                    if l < 5:
                        allgather(l)
    nc.compile()
    return nc


# ---------------------------------------------------------------- entry point

def kernel(**inputs):
    x = np.asarray(inputs["x"], np.float32)
    edge_index = np.asarray(inputs["edge_index"], np.int64)
    n, f_in = x.shape
    F = math.ceil(f_in / P) * P
    H1 = inputs["W1"].shape[1]
    H2 = inputs["W2"].shape[1]
    UH = inputs["Uw1"].shape[1]
    OUT = inputs["Uw2"].shape[1]
    import os
    tiles_pc = math.ceil(n / (NCORES * P))
    lobase = min(LOBASE, tiles_pc * P * NCORES)  # small graphs: single range
    if os.environ.get("GCN_LOBASE_OVERRIDE"):
        lobase = int(os.environ["GCN_LOBASE_OVERRIDE"])
    gblk = 2 if H1 >= 1024 else 4

    meta, m_cores, idx_cores = _plan(edge_index, n, tiles_pc, gblk, lobase)
    npc, npad = meta["npc"], meta["npad"]

    nc = _build(meta, F, H1, H2, UH, OUT, lobase)

    # shared inputs
    def f32r(a):
        return np.ascontiguousarray(np.asarray(a, np.float32))

    W1p = np.zeros((F, H1), np.float32)
    W1p[:f_in] = np.asarray(inputs["W1"], np.float32)
    shared = {"W1": W1p}
    for i in (2, 3, 4, 5):
        shared[f"W{i}"] = f32r(inputs[f"W{i}"])
    WIDTH = [H1, H2, H2, H2, H2]
    for i in range(5):
        fb = np.zeros((P, WIDTH[i]), ml_dtypes.bfloat16)
        fb[0] = np.asarray(inputs[f"b{i+1}"], np.float32).astype(ml_dtypes.bfloat16)
        shared[f"FB{i+1}"] = fb
    shared["Uw1"] = f32r(inputs["Uw1"])
    shared["Uw2"] = f32r(inputs["Uw2"])
    shared["Ub1"] = f32r(inputs["Ub1"]).reshape(UH, 1)
    shared["Ub2"] = f32r(inputs["Ub2"]).reshape(OUT, 1)
    shared["IDENT"] = np.eye(P, dtype=np.float32)
    mb = np.zeros((P, P), ml_dtypes.bfloat16)
    mb[0, :] = 1.0
    shared["MBIAS"] = mb

    xpad = np.zeros((npad, F), np.float32)
    xpad[:n, :f_in] = x
    in_maps = []
    for c in range(NCORES):
        im = dict(shared)
        im["xT"] = np.ascontiguousarray(xpad[c * npc:(c + 1) * npc].T)
        im["M"] = m_cores[c]
        im["IDXW"] = idx_cores[c]
        in_maps.append(im)

    from concourse.bass_utils import run_bass_kernel_spmd
    trace = bool(os.environ.get("GCN_TRACE"))
    res = run_bass_kernel_spmd(nc, in_maps, core_ids=list(range(NCORES)),
                               trace=trace)
    global LAST_RESULT
    LAST_RESULT = res
    out = np.empty((npad, OUT), np.float32)
    for c in range(NCORES):
        out[c * npc:(c + 1) * npc] = res.results[c]["outT"].T
    return out[:n]


# revision 19
# speedup vs baseline: 1.5154x; 1.5154x over previous
"""GCN (5x GCNConv + 2-layer head) on 8 Trainium2 NeuronCores.

Strategy:
  - Node sharding: each core owns NPC = tiles_pc*128 node rows (dst tiles).
  - Per layer: transform (x @ W, float32r matmuls) -> AllGather of h (bf16)
    -> aggregation as one-hot matmul scatter (bf16) fused with relu,
    PE-transpose and the next layer's transform.
  - Edge gather via dma_gather (SWDGE) with lo/hi source-range split
    (int16 index limit of 32768 rows per gather table slice).
"""

import math
import numpy as np
import ml_dtypes

P = 128
NCORES = 8
LOBASE = 32768  # max rows addressable by one int16 gather table slice


# ---------------------------------------------------------------- host planning

def _balance(indeg, n_nodes, n_tiles):
    """Greedy balanced assignment of nodes to tiles (capacity P each).

    Returns perm[node] = permuted row (tile*P + pos)."""
    import heapq
    order = np.argsort(-indeg, kind="stable")
    heap = [(0, 0, t) for t in range(n_tiles)]
    heapq.heapify(heap)
    perm = np.empty(n_nodes, np.int64)
    for node in order:
        load, cntt, t = heapq.heappop(heap)
        perm[node] = t * P + cntt
        if cntt + 1 < P:
            heapq.heappush(heap, (load + int(indeg[node]), cntt + 1, t))
    return perm


def _plan(edge_index, n_nodes, tiles_pc, gblk, lobase, sub_bounds, tail_gran=32):
    """Build the (schedule, per-core data) for the aggregation.

    Returns meta (identical across cores; baked into the program) and
    per-core numpy inputs (M matrices + wrapped int16 gather indices),
    plus the node permutation.

    sub_bounds: local-row boundaries of the sub-AllGathers (must contain
    lobase // NCORES when lobase < npad). h_full table layout is
    sub-AG-major: rows of sub s = [NCORES*a_s, NCORES*b_s).
    """
    npc = tiles_pc * P
    npad = NCORES * npc
    n_tiles_g = NCORES * tiles_pc

    src0 = np.concatenate([edge_index[0], np.arange(n_nodes, dtype=np.int64)])
    dst0 = np.concatenate([edge_index[1], np.arange(n_nodes, dtype=np.int64)])
    deg = np.bincount(dst0, minlength=n_nodes).astype(np.float64)
    deg[deg == 0] = 1.0
    dinv = 1.0 / np.sqrt(deg)
    norm = (dinv[src0] * dinv[dst0]).astype(np.float32)

    # balanced node -> permuted-row assignment
    perm = _balance(np.bincount(dst0, minlength=n_nodes), n_nodes, n_tiles_g)
    src = perm[src0]
    dst = perm[dst0]

    # owner-space -> table-space (sub-AG-major layout)
    sb = np.asarray(sub_bounds, np.int64)
    assert sb[0] == 0 and sb[-1] == npc
    core_of = src // npc
    loc = src % npc
    si = np.searchsorted(sb, loc, side="right") - 1
    trow = NCORES * sb[si] + core_of * (sb[si + 1] - sb[si]) + (loc - sb[si])
    if lobase < npad:
        assert lobase % NCORES == 0 and (lobase // NCORES) in set(sb.tolist()), \
            (lobase, sb)
    rng_ = (trow >= lobase).astype(np.int64)
    idx_val_all = (trow - rng_ * lobase).astype(np.int64)
    assert idx_val_all.max() < 32768

    gtile = dst // P
    core = gtile // tiles_pc
    ltile = gtile % tiles_pc
    dloc = dst % P

    # counts per (core, ltile, range) -> uniform structure via max over cores
    cnt = np.zeros((NCORES, tiles_pc, 2), np.int64)
    np.add.at(cnt, (core, ltile, rng_), 1)
    cmax = cnt.max(axis=0)                      # [tiles_pc, 2]
    nfull = cmax // P                           # full 128-slots per (t, r)
    tail = cmax - nfull * P
    tailsz = ((tail + tail_gran - 1) // tail_gran) * tail_gran  # region size

    blocks = [(b, min(b + gblk, tiles_pc)) for b in range(0, tiles_pc, gblk)]

    # slot/chunk layout. slots are gather units of 128 rows; chunks are
    # matmul units (one (tile, slot) pair with its own M matrix).
    full_slot0 = np.zeros((tiles_pc, 2), np.int64)   # first full slot of (t,r)
    tail_slot = np.zeros((tiles_pc, 2), np.int64)    # shared slot of the tail
    tail_off = np.zeros((tiles_pc, 2), np.int64)     # offset inside that slot
    gathers = []     # per blk: list of (r, slot0, nslots, col0)
    chunks_per_tile = [[] for _ in range(tiles_pc)]  # (chunk_id, slot)
    s = 0
    col = 0
    n_chunks = 0
    chunk_of = {}    # (t, r, j) -> chunk id  (j = full index or -1 for tail)
    for (t0, t1) in blocks:
        gspec = []
        for r in (0, 1):
            s0, c0 = s, col
            for t in range(t0, t1):
                full_slot0[t, r] = s
                for j in range(int(nfull[t, r])):
                    chunk_of[(t, r, j)] = n_chunks
                    chunks_per_tile[t].append((n_chunks, s))
                    n_chunks += 1
                    s += 1
            # pack tail regions of this (blk, r) into shared slots
            cur_off = P  # force new slot on first region
            for t in range(t0, t1):
                tsz = int(tailsz[t, r])
                if tsz == 0:
                    continue
                if cur_off + tsz > P:
                    s += 1
                    cur_off = 0
                tail_slot[t, r] = s - 1
                tail_off[t, r] = cur_off
                chunk_of[(t, r, -1)] = n_chunks
                chunks_per_tile[t].append((n_chunks, s - 1))
                n_chunks += 1
                cur_off += tsz
            nsl = s - s0
            gspec.append((r, s0, nsl, c0))
            col += nsl * P // 16
        gathers.append(gspec)
    s_total = s
    idx_cols = col
    blk_slots = [gs[0][2] + gs[1][2] for gs in gathers]

    meta = dict(
        tiles_pc=tiles_pc, npc=npc, npad=npad, blocks=blocks, gathers=gathers,
        chunks_per_tile=chunks_per_tile, s_total=s_total, idx_cols=idx_cols,
        blk_slots=blk_slots, n_chunks=n_chunks, sub_bounds=list(sub_bounds),
    )

    # ---- per-edge placement
    order = np.lexsort((idx_val_all, rng_, ltile, core))
    so, no_, co, to, ro, do, iv = (a[order] for a in
                                   (src, norm, core, ltile, rng_, dloc,
                                    idx_val_all))
    key = (co * tiles_pc + to) * 2 + ro
    grp_start = np.searchsorted(key, np.arange(n_tiles_g * 2, dtype=np.int64),
                                side="left")
    pos = np.arange(len(so)) - grp_start[key]

    nfull_e = nfull[to, ro]
    is_tail = pos >= nfull_e * P
    # slot and in-slot position of each edge
    slot_e = np.where(is_tail, tail_slot[to, ro],
                      full_slot0[to, ro] + pos // P)
    pos_e = np.where(is_tail, tail_off[to, ro] + (pos - nfull_e * P), pos % P)
    # chunk of each edge
    cid_full = np.zeros((tiles_pc, 2, max(int(nfull.max()), 1)), np.int64)
    cid_tail = np.zeros((tiles_pc, 2), np.int64)
    for (t, r, j), cid in chunk_of.items():
        if j < 0:
            cid_tail[t, r] = cid
        else:
            cid_full[t, r, j] = cid
    chunk_e = np.where(is_tail, cid_tail[to, ro],
                       cid_full[to, ro, np.minimum(pos // P,
                                                   cid_full.shape[2] - 1)])

    colbase = np.zeros(s_total, np.int64)
    gslot0 = np.zeros(s_total, np.int64)
    for gspec in gathers:
        for (r, s0, nsl, c0) in gspec:
            colbase[s0:s0 + nsl] = c0
            gslot0[s0:s0 + nsl] = s0

    m_cores, idx_cores = [], []
    for c in range(NCORES):
        sel = co == c
        M = np.zeros((n_chunks, P, P), ml_dtypes.bfloat16)
        M[chunk_e[sel], pos_e[sel], do[sel]] = no_[sel].astype(ml_dtypes.bfloat16)
        idxw = np.zeros((16, idx_cols), np.int16)
        j = (slot_e[sel] - gslot0[slot_e[sel]]) * P + pos_e[sel]
        idxw[j % 16, colbase[slot_e[sel]] + j // 16] = iv[sel].astype(np.int16)
        m_cores.append(M)
        idx_cores.append(np.tile(idxw, (8, 1)))
    return meta, m_cores, idx_cores, perm


# ---------------------------------------------------------------- device kernel

def _build(meta, F, H1, H2, UH, OUT, lobase):
    import concourse.bass as bass
    import concourse.bacc as bacc
    import concourse.tile as tile
    import concourse.mybir as mybir

    dt = mybir.dt
    AF = mybir.ActivationFunctionType
    tiles_pc = meta["tiles_pc"]
    npc, npad = meta["npc"], meta["npad"]
    s_total, idx_cols = meta["s_total"], meta["idx_cols"]
    WIDTH = [H1, H2, H2, H2, H2]          # width of h_l, l=1..5
    WMAT_IN = [F, H1, H2, H2, H2]         # contraction dim of W_l

    nc = bacc.Bacc("TRN2", target_bir_lowering=False, debug=False,
                   num_devices=NCORES)

    xT = nc.dram_tensor("xT", [F, npc], dt.float32r, kind="ExternalInput").ap()
    Ws = [nc.dram_tensor(f"W{l+1}", [WMAT_IN[l], WIDTH[l]], dt.float32r,
                         kind="ExternalInput").ap() for l in range(5)]
    FBs = [nc.dram_tensor(f"FB{l+1}", [P, WIDTH[l]], dt.bfloat16,
                          kind="ExternalInput").ap() for l in range(5)]
    Uw1 = nc.dram_tensor("Uw1", [H2, UH], dt.float32r, kind="ExternalInput").ap()
    Uw2 = nc.dram_tensor("Uw2", [UH, OUT], dt.float32r, kind="ExternalInput").ap()
    Ub1 = nc.dram_tensor("Ub1", [UH, 1], dt.float32, kind="ExternalInput").ap()
    Ub2 = nc.dram_tensor("Ub2", [OUT, 1], dt.float32, kind="ExternalInput").ap()
    n_chunks = meta["n_chunks"]
    Mt = nc.dram_tensor("M", [n_chunks, P, P], dt.bfloat16, kind="ExternalInput").ap()
    IDXW = nc.dram_tensor("IDXW", [P, idx_cols], dt.int16, kind="ExternalInput").ap()
    IDENT = nc.dram_tensor("IDENT", [P, P], dt.float32r, kind="ExternalInput").ap()
    MBIAS = nc.dram_tensor("MBIAS", [P, P], dt.bfloat16, kind="ExternalInput").ap()
    outT = nc.dram_tensor("outT", [OUT, npc], dt.float32, kind="ExternalOutput").ap()

    with tile.TileContext(nc) as tc:
        with tc.tile_pool(name="res", bufs=1) as res, \
             tc.tile_pool(name="dram", bufs=1, space="DRAM") as dram:
            # resident tiles
            m_sb = res.tile([P, n_chunks * P], dt.bfloat16)
            nc.sync.dma_start(
                m_sb[:].rearrange("e (s d) -> e s d", d=P),
                Mt[:].rearrange("s e d -> e s d"))
            idx_sb = res.tile([P, idx_cols], dt.int16)
            nc.sync.dma_start(idx_sb[:], IDXW[:])
            ident_sb = res.tile([P, P], dt.float32r)
            nc.sync.dma_start(ident_sb[:], IDENT[:])
            mbias_sb = res.tile([P, P], dt.bfloat16)
            nc.sync.dma_start(mbias_sb[:], MBIAS[:])
            fb_sb = []
            for l in range(5):
                t = res.tile([P, WIDTH[l]], dt.bfloat16, name=f"fb{l}")
                nc.sync.dma_start(t[:], FBs[l][:])
                fb_sb.append(t)
            uw1_sb = res.tile([P, (H2 // P) * UH], dt.float32r)
            nc.sync.dma_start(uw1_sb[:].rearrange("p (k u) -> p k u", u=UH),
                              Uw1[:].rearrange("(k p) u -> p k u", p=P))
            uw2_sb = res.tile([UH, OUT], dt.float32r)
            nc.sync.dma_start(uw2_sb[:], Uw2[:])
            ub1_sb = res.tile([UH, 1], dt.float32)
            nc.sync.dma_start(ub1_sb[:], Ub1[:])
            ub2_sb = res.tile([OUT, 1], dt.float32)
            nc.sync.dma_start(ub2_sb[:], Ub2[:])

            h_own = [dram.tile([npc, WIDTH[l]], dt.bfloat16, name=f"h_own{l}")
                     for l in range(5)]
            h_full = [dram.tile([npad, WIDTH[l]], dt.bfloat16, name=f"h_full{l}")
                      for l in range(5)]

            sb_ = meta["sub_bounds"]

            def allgather(l):
                for a, b in zip(sb_[:-1], sb_[1:]):
                    nc.gpsimd.collective_compute(
                        "AllGather", mybir.AluOpType.bypass,
                        replica_groups=[list(range(NCORES))],
                        ins=[h_own[l][a:b, :]],
                        outs=[h_full[l][NCORES * a:NCORES * b, :]])

            # ---------------- stage 0: transform layer 1 (x @ W1)
            kt1 = F // P
            GW = 4  # tiles per xT column group
            with tc.tile_pool(name="s0w", bufs=1) as s0w, \
                 tc.tile_pool(name="s0x", bufs=2) as s0x, \
                 tc.tile_pool(name="s0ps", bufs=2, space="PSUM") as s0ps, \
                 tc.tile_pool(name="s0h", bufs=3) as s0h:
                w1_sb = s0w.tile([P, kt1 * H1], dt.float32r)
                nc.sync.dma_start(
                    w1_sb[:].rearrange("p (k h) -> p k h", h=H1),
                    Ws[0][:].rearrange("(k p) h -> p k h", p=P))
                w1r = w1_sb[:].rearrange("p (k h) -> p k h", h=H1)
                for g0 in range(0, tiles_pc, GW):
                    g1 = min(g0 + GW, tiles_pc)
                    gw = (g1 - g0) * P
                    xg = s0x.tile([P, kt1 * GW * P], dt.float32r, tag="xg")
                    nc.sync.dma_start(
                        xg[:, :kt1 * gw].rearrange("p (k c) -> p k c", c=gw),
                        xT[:, g0 * P:g1 * P].rearrange("(k p) c -> p k c", p=P))
                    xgr = xg[:, :kt1 * gw].rearrange("p (k c) -> p k c", c=gw)
                    for t in range(g0, g1):
                        tl = (t - g0) * P
                        ps = s0ps.tile([P, H1], dt.float32, space="PSUM", tag="ps")
                        nreg0 = max(H1 // 512, 1)
                        rw0 = min(H1, 512)
                        for k in range(kt1):
                            for nh in range(nreg0):
                                nc.tensor.matmul(
                                    ps[:, nh * rw0:(nh + 1) * rw0],
                                    lhsT=xgr[:, k, tl:tl + P],
                                    rhs=w1r[:, k, nh * rw0:(nh + 1) * rw0],
                                    start=(k == 0), stop=(k == kt1 - 1))
                        hb = s0h.tile([P, H1], dt.bfloat16, tag="hb")
                        nc.scalar.activation(hb[:], ps[:], AF.Copy)
                        nc.sync.dma_start(h_own[0][t * P:(t + 1) * P, :], hb[:])
            allgather(0)

            # ---------------- stages l=1..5: agg + (next transform | head)
            for l in range(1, 6):
                w_in = WIDTH[l - 1]
                ktin = w_in // P
                nreg = max(w_in // 512, 1)
                rwid = min(w_in, 512)
                w_out = WIDTH[l] if l < 5 else None
                with tc.tile_pool(name=f"s{l}w", bufs=1) as sw, \
                     tc.tile_pool(name=f"s{l}g", bufs=2) as sg, \
                     tc.tile_pool(name=f"s{l}ag", bufs=2, space="PSUM") as agp, \
                     tc.tile_pool(name=f"s{l}tr", bufs=2, space="PSUM") as trp, \
                     tc.tile_pool(name=f"s{l}tf", bufs=2, space="PSUM") as tfp, \
                     tc.tile_pool(name=f"s{l}sb", bufs=3) as sb:
                    if l < 5:
                        wn_sb = sw.tile([P, ktin * w_out], dt.float32r)
                        nc.sync.dma_start(
                            wn_sb[:].rearrange("p (k h) -> p k h", h=w_out),
                            Ws[l][:].rearrange("(k p) h -> p k h", p=P))
                        wnr = wn_sb[:].rearrange("p (k h) -> p k h", h=w_out)
                    uw1r = uw1_sb[:].rearrange("p (k u) -> p k u", u=UH)

                    for bi, (t0, t1) in enumerate(meta["blocks"]):
                        nsl = meta["blk_slots"][bi]
                        gt = sg.tile([P, nsl * w_in], dt.bfloat16, tag="gt")
                        g3 = gt[:].rearrange("p (s e) -> p s e", e=w_in)
                        for (r, s0, ns_r, c0) in meta["gathers"][bi]:
                            if ns_r == 0:
                                continue
                            pos0 = s0 - meta["gathers"][bi][0][1]
                            tslice = (h_full[l - 1][0:lobase, :] if r == 0
                                      else h_full[l - 1][lobase:npad, :])
                            nc.gpsimd.dma_gather(
                                out_ap=g3[:, pos0:pos0 + ns_r, :],
                                in_ap=tslice,
                                idxs_ap=idx_sb[:, c0:c0 + ns_r * P // 16],
                                num_idxs=ns_r * P, num_idxs_reg=ns_r * P,
                                elem_size=w_in)
                        for t in range(t0, t1):
                            ps = agp.tile([P, w_in], dt.float32, space="PSUM",
                                          tag="agps")
                            chunks = meta["chunks_per_tile"][t]
                            for nh in range(nreg):
                                nc.tensor.matmul(
                                    ps[:, nh * rwid:(nh + 1) * rwid],
                                    lhsT=mbias_sb[:],
                                    rhs=fb_sb[l - 1][:, nh * rwid:(nh + 1) * rwid],
                                    start=True, stop=False)
                            blk_s0 = meta["gathers"][bi][0][1]
                            for ci, (cid, s) in enumerate(chunks):
                                sl = s - blk_s0
                                last = ci == len(chunks) - 1
                                for nh in range(nreg):
                                    nc.tensor.matmul(
                                        ps[:, nh * rwid:(nh + 1) * rwid],
                                        lhsT=m_sb[:, cid * P:(cid + 1) * P],
                                        rhs=g3[:, sl, nh * rwid:(nh + 1) * rwid],
                                        start=False, stop=last)
                            xr = sb.tile([P, w_in], dt.float32r, tag="xr")
                            nc.scalar.activation(xr[:], ps[:], AF.Relu)
                            xt = sb.tile([P, w_in], dt.float32r, tag="xt")
                            for k in range(ktin):
                                tp = trp.tile([P, P], dt.float32r, space="PSUM",
                                              tag="trps")
                                nc.tensor.transpose(
                                    tp[:], xr[:, k * P:(k + 1) * P], ident_sb[:])
                                nc.vector.tensor_copy(xt[:, k * P:(k + 1) * P], tp[:])
                            if l < 5:
                                ps2 = tfp.tile([P, w_out], dt.float32,
                                               space="PSUM", tag="tfps")
                                for k in range(ktin):
                                    nc.tensor.matmul(
                                        ps2[:], lhsT=xt[:, k * P:(k + 1) * P],
                                        rhs=wnr[:, k, :],
                                        start=(k == 0), stop=(k == ktin - 1))
                                hb = sb.tile([P, w_out], dt.bfloat16, tag="hb")
                                nc.scalar.activation(hb[:], ps2[:], AF.Copy)
                                nc.sync.dma_start(
                                    h_own[l][t * P:(t + 1) * P, :], hb[:])
                            else:
                                psh = tfp.tile([UH, P], dt.float32, space="PSUM",
                                               tag="h1ps")
                                for k in range(ktin):
                                    nc.tensor.matmul(
                                        psh[:], lhsT=uw1r[:, k, :],
                                        rhs=xt[:, k * P:(k + 1) * P],
                                        start=(k == 0), stop=(k == ktin - 1))
                                hh = sb.tile([UH, P], dt.float32r, tag="hh")
                                nc.scalar.activation(hh[:], psh[:], AF.Relu,
                                                     bias=ub1_sb[:])
                                pso = trp.tile([OUT, P], dt.float32, space="PSUM",
                                               tag="h2ps")
                                nc.tensor.matmul(pso[:], lhsT=uw2_sb[:],
                                                 rhs=hh[:], start=True, stop=True)
                                ob = sb.tile([OUT, P], dt.float32, tag="ob")
                                nc.vector.tensor_tensor(
                                    ob[:], pso[:], ub2_sb[:].to_broadcast([OUT, P]),
                                    op=mybir.AluOpType.add)
                                nc.sync.dma_start(outT[:, t * P:(t + 1) * P], ob[:])
                    if l < 5:
                        allgather(l)
    nc.compile()
    return nc


# ---------------------------------------------------------------- entry point

def kernel(**inputs):
    x = np.asarray(inputs["x"], np.float32)
    edge_index = np.asarray(inputs["edge_index"], np.int64)
    n, f_in = x.shape
    F = math.ceil(f_in / P) * P
    H1 = inputs["W1"].shape[1]
    H2 = inputs["W2"].shape[1]
    UH = inputs["Uw1"].shape[1]
    OUT = inputs["Uw2"].shape[1]
    import os
    tiles_pc = math.ceil(n / (NCORES * P))
    npc_ = tiles_pc * P
    lobase = min(LOBASE, npc_ * NCORES)  # small graphs: single range
    if os.environ.get("GCN_LOBASE_OVERRIDE"):
        lobase = int(os.environ["GCN_LOBASE_OVERRIDE"])
    gblk = 2 if H1 >= 1024 else 4

    # sub-AllGather boundaries (local rows): include lobase/NCORES, plus an
    # extra split of the lo range for compute/comm overlap.
    sub = {0, npc_}
    if lobase < npc_ * NCORES:
        lb = lobase // NCORES
        sub.add(lb)
        if lb % (2 * P) == 0:
            sub.add(lb // 2)
    sub_bounds = sorted(sub)

    meta, m_cores, idx_cores, perm = _plan(edge_index, n, tiles_pc, gblk,
                                           lobase, sub_bounds)
    npc, npad = meta["npc"], meta["npad"]

    nc = _build(meta, F, H1, H2, UH, OUT, lobase)

    # shared inputs
    def f32r(a):
        return np.ascontiguousarray(np.asarray(a, np.float32))

    W1p = np.zeros((F, H1), np.float32)
    W1p[:f_in] = np.asarray(inputs["W1"], np.float32)
    shared = {"W1": W1p}
    for i in (2, 3, 4, 5):
        shared[f"W{i}"] = f32r(inputs[f"W{i}"])
    WIDTH = [H1, H2, H2, H2, H2]
    for i in range(5):
        fb = np.zeros((P, WIDTH[i]), ml_dtypes.bfloat16)
        fb[0] = np.asarray(inputs[f"b{i+1}"], np.float32).astype(ml_dtypes.bfloat16)
        shared[f"FB{i+1}"] = fb
    shared["Uw1"] = f32r(inputs["Uw1"])
    shared["Uw2"] = f32r(inputs["Uw2"])
    shared["Ub1"] = f32r(inputs["Ub1"]).reshape(UH, 1)
    shared["Ub2"] = f32r(inputs["Ub2"]).reshape(OUT, 1)
    shared["IDENT"] = np.eye(P, dtype=np.float32)
    mb = np.zeros((P, P), ml_dtypes.bfloat16)
    mb[0, :] = 1.0
    shared["MBIAS"] = mb

    xpad = np.zeros((npad, F), np.float32)
    xpad[perm, :f_in] = x
    in_maps = []
    for c in range(NCORES):
        im = dict(shared)
        im["xT"] = np.ascontiguousarray(xpad[c * npc:(c + 1) * npc].T)
        im["M"] = m_cores[c]
        im["IDXW"] = idx_cores[c]
        in_maps.append(im)

    from concourse.bass_utils import run_bass_kernel_spmd
    trace = bool(os.environ.get("GCN_TRACE"))
    res = run_bass_kernel_spmd(nc, in_maps, core_ids=list(range(NCORES)),
                               trace=trace)
    global LAST_RESULT
    LAST_RESULT = res
    out = np.empty((npad, OUT), np.float32)
    for c in range(NCORES):
        out[c * npc:(c + 1) * npc] = res.results[c]["outT"].T
    return out[perm]
